# revision 3
# baseline (speedup 1.0000x reference)
"""Trainium2 Bass kernel for nn_DiffusionGraphConv_89936615178296.

out = relu(A_hat @ (x @ (W1+W2)) + b1 + b2), A_hat = D^-1/2 (A+I) D^-1/2.

Reformulation: out = relu(dinv * (Agg(dinv*x) @ W) + b), W = W1+W2,
dinv = rsqrt(1 + in_degree); the per-edge norm dinv[src]*dinv[dst] is
separable so per-edge work is a pure gather + segment-sum of x' = dinv*x.

8 cores, no collectives. Nodes sharded round-robin in degree-sorted order.
Each core:
  P0: builds the fp16 table x' = dinv*x for the full graph in its local HBM.
  P1: per destination tile group, one batched dma_gather (InstDMAGatherAnt,
      int16 element indices = src>>2, each 256B element = 4 packed fp16 rows)
      pulls all slots into a [128, cols, 128] grid; a host-built one-hot
      phase mask (src&3) zeroes the 3 unwanted rows per slot; one 4D-AP
      tensor_reduce folds (slot, phase) per tile; dinv scale; batched
      transpose + block-diag matmul with W on the tensor engine; relu;
      final DMA writes the output shard.

Host does integer index prep only (degree counts, shard permutation, slot
layout, int16 gather indices, 0/1 phase masks); all f32 arithmetic runs on
device.
"""

import numpy as np
from contextlib import ExitStack

import concourse.bass as bass
import concourse.bacc as bacc
import concourse.tile as tile
from concourse import mybir
from concourse.masks import make_identity
from concourse.bass_utils import run_bass_kernel_spmd

N, E, C = 100000, 1600000, 32
M = 8
P = 128
TILES = 98                  # 98*128 = 12544 rows per core >= 12500
TABLE_STRIPS = 783          # 783*128 = 100224 >= N+4 (zero rows 100000..100003)
TABLE_ROWS = TABLE_STRIPS * P
ZERO_ELEM = 25000           # element covering rows 100000..100003 (all zero)
NELEM = TABLE_ROWS * C // 128  # gatherable 256B elements in the table
G_STRIP = 64                # table build strip width
NT = 4                      # tiles per group
CALL_COLS = 120             # max grid columns per dma_gather call (15360 idxs)


def _host_prep(x, edge_index):
    src = np.asarray(edge_index[0], dtype=np.int64)
    dst = np.asarray(edge_index[1], dtype=np.int64)
    x = np.asarray(x, dtype=np.float32)

    deg = np.bincount(dst, minlength=N).astype(np.int64) + 1

    order = np.argsort(-deg, kind="stable")
    ar = np.arange(N)
    core_of = np.empty(N, np.int64)
    pos_of = np.empty(N, np.int64)
    core_of[order] = ar % M
    pos_of[order] = ar // M

    degs_sorted = deg[order]
    # per-tile edge-slot count (self-loop handled via x_shard), >= 1
    Kt = [int(degs_sorted[t * P * M]) - 1 if t * P * M < N else 1 for t in range(TILES)]
    Kt = [max(1, k) for k in Kt]

    # groups of NT tiles with uniform K = max tile K in group
    groups = []  # (g0, g1, KG, colbase)
    col = 0
    for g0 in range(0, TILES, NT):
        g1 = min(g0 + NT, TILES)
        KG = max(Kt[g0:g1])
        groups.append((g0, g1, KG, col))
        col += KG * (g1 - g0)
    SG = col  # total grid columns per core

    # gather calls: split groups at CALL_COLS boundaries
    # call = (col_start, ncols, o16) ; o16 = offset into idx16 array cols
    calls = []
    o16 = 0
    for (g0, g1, KG, cb) in groups:
        ncols_g = KG * (g1 - g0)
        cs = 0
        while cs < ncols_g:
            cc = min(CALL_COLS, ncols_g - cs)
            calls.append((cb + cs, cc, o16))
            o16 += cc * P // 16
            cs += cc
    TOT16 = o16

    t_of = pos_of >> 7
    p_of = pos_of & 127

    # global column of tile t slot k:  colbase(group) + (t%NT)*KG + k
    gidx = np.arange(TILES) // NT
    KG_of_t = np.array([groups[g][2] for g in gidx])
    colbase_of_t = np.array([groups[g][3] for g in gidx]) + (
        np.arange(TILES) - gidx * NT
    ) * KG_of_t

    # CSR by destination: rank of edge within its dst
    eorder = np.argsort(dst, kind="stable")
    src_sorted = src[eorder]
    dstn = dst[eorder]
    starts = np.concatenate([[0], np.cumsum(deg - 1)])
    rank = np.arange(E, dtype=np.int64) - starts[dstn]

    ecore = core_of[dstn]
    gcol = colbase_of_t[t_of[dstn]] + rank
    flat = gcol * P + p_of[dstn]            # slot id within core, col-major

    # int16 element indices (src>>2), pad = ZERO_ELEM
    A = np.full((M, SG * P), ZERO_ELEM, np.int16)
    A[ecore, flat] = (src_sorted >> 2).astype(np.int16)
    # phase mask fp16 [M, P, 4*SG]: 1.0 at (p, 4*gcol + (src&3))
    mask = np.zeros((M, P, 4 * SG), np.float16)
    mask[ecore, p_of[dstn], 4 * gcol + (src_sorted & 3)] = 1.0

    # wrap each call's indices: [16, n/16] with [l, s] = arr[s*16+l], tile x8
    idx16 = np.zeros((M, P, TOT16), np.int16)
    for (c0, cc, o16c) in calls:
        n = cc * P
        sl = A[:, c0 * P : c0 * P + n]                       # [M, n]
        wr = sl.reshape(M, n // 16, 16).transpose(0, 2, 1)   # [M, 16, n/16]
        idx16[:, :, o16c : o16c + n // 16] = np.tile(wr, (1, 8, 1))

    # per-core own-shard features, [P, TILES, C], zero on pad rows
    x_shard = np.zeros((M, P, TILES, C), np.float32)
    x_shard[core_of, p_of, t_of] = x

    deg_pad = np.ones(TABLE_ROWS, np.float32)
    deg_pad[:N] = deg
    deg_arr = np.ones((P, TABLE_STRIPS), np.float32)
    colpos = 0
    for s0 in range(0, TABLE_STRIPS, G_STRIP):
        g = min(G_STRIP, TABLE_STRIPS - s0)
        blk = deg_pad[s0 * P : (s0 + g) * P].reshape(P, g)
        deg_arr[:, colpos : colpos + g] = blk
        colpos += g

    deg_shard = np.ones((M, P, TILES), np.float32)
    deg_shard[core_of, p_of, t_of] = deg.astype(np.float32)

    x_pad = np.zeros((TABLE_ROWS, C), np.float32)
    x_pad[:N] = x

    return dict(
        groups=groups, calls=calls, SG=SG, TOT16=TOT16, idx16=idx16,
        mask=mask, deg_arr=deg_arr, deg_shard=deg_shard, x_pad=x_pad,
        core_of=core_of, pos_of=pos_of, x_shard=x_shard,
    )


def _build_program(groups, calls, SG, TOT16, bias_zero):
    nc = bacc.Bacc("TRN2", target_bir_lowering=False, debug=False, num_devices=M)
    f32, f16, i16 = mybir.dt.float32, mybir.dt.float16, mybir.dt.int16

    x_d = nc.dram_tensor("x_pad", [TABLE_ROWS, C], f32, kind="ExternalInput")
    dega_d = nc.dram_tensor("deg_arr", [P, TABLE_STRIPS], f32, kind="ExternalInput")
    degs_d = nc.dram_tensor("deg_shard", [P, TILES], f32, kind="ExternalInput")
    idx_d = nc.dram_tensor("idx16", [P, TOT16], i16, kind="ExternalInput")
    mask_d = nc.dram_tensor("mask", [P, 4 * SG], f16, kind="ExternalInput")
    xs_d = nc.dram_tensor("x_shard", [P, TILES * C], f32, kind="ExternalInput")
    w1_d = nc.dram_tensor("W1", [C, C], f32, kind="ExternalInput")
    w2_d = nc.dram_tensor("W2", [C, C], f32, kind="ExternalInput")
    b1_d = nc.dram_tensor("b1", [C], f32, kind="ExternalInput")
    b2_d = nc.dram_tensor("b2", [C], f32, kind="ExternalInput")
    table_d = nc.dram_tensor("table", [TABLE_ROWS, C], f16, kind="Internal")
    out_d = nc.dram_tensor("out", [P, TILES * C], f32, kind="ExternalOutput")

    with tile.TileContext(nc) as tc, ExitStack() as ctx:
        singles = ctx.enter_context(tc.tile_pool(name="singles", bufs=1))
        xpool = ctx.enter_context(tc.tile_pool(name="xin", bufs=3))
        tpool = ctx.enter_context(tc.tile_pool(name="tout", bufs=3))
        gpool = ctx.enter_context(tc.tile_pool(name="gather", bufs=2))
        apool = ctx.enter_context(tc.tile_pool(name="agg", bufs=3))
        tspool = ctx.enter_context(tc.tile_pool(name="trsb", bufs=3))
        pst = ctx.enter_context(tc.tile_pool(name="pst", bufs=3, space="PSUM"))
        psm = ctx.enter_context(tc.tile_pool(name="psm", bufs=3, space="PSUM"))

        # ---- singles ----
        w4a = singles.tile([P, P], f32)
        w4b = singles.tile([P, P], f32)
        nc.vector.memset(w4a[:], 0.0)
        nc.gpsimd.memset(w4b[:], 0.0)
        for t in range(NT):
            sl = slice(t * C, (t + 1) * C)
            nc.sync.dma_start(out=w4a[sl, sl], in_=w1_d.ap())
            nc.sync.dma_start(out=w4b[sl, sl], in_=w2_d.ap())
        nc.vector.tensor_add(out=w4a[:], in0=w4a[:], in1=w4b[:])

        if not bias_zero:
            b1_sb = singles.tile([P, C], f32)
            b2_sb = singles.tile([P, C], f32)
            nc.sync.dma_start(
                out=b1_sb[:], in_=bass.AP(tensor=b1_d, offset=0, ap=[[0, P], [1, C]])
            )
            nc.sync.dma_start(
                out=b2_sb[:], in_=bass.AP(tensor=b2_d, offset=0, ap=[[0, P], [1, C]])
            )
            bsum_sb = singles.tile([P, C], f32)
            nc.vector.tensor_add(out=bsum_sb[:], in0=b1_sb[:], in1=b2_sb[:])

        ident = singles.tile([P, P], f32)
        make_identity(nc, ident[:])

        idx_sb = singles.tile([P, TOT16], i16)
        nc.sync.dma_start(out=idx_sb[:], in_=idx_d.ap())
        mask_sb = singles.tile([P, 4 * SG], f16)
        nc.sync.dma_start(out=mask_sb[:], in_=mask_d.ap())

        dinv_all = singles.tile([P, TABLE_STRIPS], f32)
        nc.sync.dma_start(out=dinv_all[:], in_=dega_d.ap())
        nc.scalar.sqrt(out=dinv_all[:], in_=dinv_all[:])
        nc.vector.reciprocal(out=dinv_all[:], in_=dinv_all[:])

        dinv_sh = singles.tile([P, TILES], f32)
        nc.sync.dma_start(out=dinv_sh[:], in_=degs_d.ap())
        nc.scalar.sqrt(out=dinv_sh[:], in_=dinv_sh[:])
        nc.vector.reciprocal(out=dinv_sh[:], in_=dinv_sh[:])

        out_all = singles.tile([P, TILES * C], f32)

        # self-loop contribution x' = dinv * x for own shard rows
        xs_all = singles.tile([P, TILES * C], f32)
        nc.sync.dma_start(out=xs_all[:], in_=xs_d.ap())
        nc.vector.tensor_tensor(
            out=xs_all[:].rearrange("p (t c) -> p t c", c=C),
            in0=xs_all[:].rearrange("p (t c) -> p t c", c=C),
            in1=dinv_sh[:].to_broadcast([P, TILES, C]),
            op=mybir.AluOpType.mult,
        )

        # ---- P0: table build, partition-contiguous strips ----
        colpos = 0
        for s0 in range(0, TABLE_STRIPS, G_STRIP):
            g = min(G_STRIP, TABLE_STRIPS - s0)
            x_ap = bass.AP(
                tensor=x_d, offset=s0 * P * C,
                ap=[[g * C, P], [C, g], [1, C]],
            )
            t_ap = bass.AP(
                tensor=table_d, offset=s0 * P * C,
                ap=[[g * C, P], [C, g], [1, C]],
            )
            x_sb = xpool.tile([P, G_STRIP, C], f32, tag="x")
            nc.sync.dma_start(out=x_sb[:, :g, :], in_=x_ap)
            t_sb = tpool.tile([P, G_STRIP, C], f16, tag="t")
            nc.vector.tensor_tensor(
                out=t_sb[:, :g, :],
                in0=x_sb[:, :g, :],
                in1=dinv_all[:, colpos : colpos + g].to_broadcast([P, g, C]),
                op=mybir.AluOpType.mult,
            )
            nc.sync.dma_start(out=t_ap, in_=t_sb[:, :g, :])
            colpos += g

        # gather source: 256B elements = 4 packed fp16 rows
        table_elems = bass.AP(tensor=table_d, offset=0, ap=[[128, NELEM], [1, 128]])

        # ---- P1 ----
        maxcols = max(KG * (g1 - g0) for g0, g1, KG, _ in groups)
        call_i = 0
        for (g0, g1, KG, cb) in groups:
            nt = g1 - g0
            ncols = KG * nt
            gbuf = gpool.tile([P, maxcols * 128], f16, tag="gbuf")
            # one (or few) batched gathers for the whole group
            cdone = 0
            while cdone < ncols:
                c0, cc, o16c = calls[call_i]
                assert c0 == cb + cdone, (c0, cb, cdone)
                nc.gpsimd.dma_gather(
                    out_ap=gbuf[:, cdone * 128 : (cdone + cc) * 128].rearrange(
                        "p (g e) -> p g e", e=128
                    ),
                    in_ap=table_elems,
                    idxs_ap=idx_sb[:, o16c : o16c + cc * P // 16],
                    num_idxs=cc * P,
                    num_idxs_reg=cc * P,
                    elem_size=128,
                    elem_step=128,
                    single_packet=False,
                )
                call_i += 1
                cdone += cc
            # phase select: grid *= one-hot mask over the 4 rows per element
            nc.vector.tensor_tensor(
                out=gbuf[:, : ncols * 128].rearrange(
                    "p (s m c) -> p s m c", m=4, c=C
                ),
                in0=gbuf[:, : ncols * 128].rearrange(
                    "p (s m c) -> p s m c", m=4, c=C
                ),
                in1=mask_sb[:, 4 * cb : 4 * (cb + ncols)]
                .rearrange("p (s m) -> p s m", m=4)
                .to_broadcast([P, ncols, 4, C]),
                op=mybir.AluOpType.mult,
            )
            # fold (slot, phase) per tile: [p, t, c, j=4K] -> [p, t, c]
            agg = apool.tile([P, NT * C], f32, tag="agg")
            nc.vector.tensor_reduce(
                out=agg[:, : nt * C],
                in_=gbuf[:, : ncols * 128].rearrange(
                    "p (t j c) -> p t c j", t=nt, c=C
                ),
                axis=mybir.AxisListType.X,
                op=mybir.AluOpType.add,
            )
            # add self-loop term
            nc.vector.tensor_add(
                out=agg[:, : nt * C],
                in0=agg[:, : nt * C],
                in1=xs_all[:, g0 * C : g1 * C],
            )
            # dinv scale (per node row)
            nc.vector.tensor_tensor(
                out=agg[:, : nt * C].rearrange("p (t c) -> p t c", c=C),
                in0=agg[:, : nt * C].rearrange("p (t c) -> p t c", c=C),
                in1=dinv_sh[:, g0:g1].to_broadcast([P, nt, C]),
                op=mybir.AluOpType.mult,
            )
            # batched transpose + block-diag matmul
            trps = pst.tile([P, P], f32, tag="trps")
            nc.tensor.transpose(
                out=trps[: nt * C, :], in_=agg[:, : nt * C], identity=ident[:]
            )
            aggdT = tspool.tile([P, P], f32, tag="aggdT")
            nc.scalar.copy(out=aggdT[: nt * C, :], in_=trps[: nt * C, :])
            mm = psm.tile([P, P], f32, tag="mm")
            nc.tensor.matmul(
                out=mm[:, : nt * C],
                lhsT=aggdT[: nt * C, :],
                rhs=w4a[: nt * C, : nt * C],
                start=True, stop=True,
            )
            osl = out_all[:, g0 * C : g1 * C]
            if bias_zero:
                nc.scalar.activation(
                    out=osl, in_=mm[:, : nt * C],
                    func=mybir.ActivationFunctionType.Relu,
                )
            else:
                for ti in range(nt):
                    nc.vector.tensor_add(
                        out=osl[:, ti * C : (ti + 1) * C],
                        in0=mm[:, ti * C : (ti + 1) * C],
                        in1=bsum_sb[:],
                    )
                nc.scalar.activation(
                    out=osl, in_=osl, func=mybir.ActivationFunctionType.Relu
                )

        nc.sync.dma_start(out=out_d.ap(), in_=out_all[:])

    nc.compile()
    return nc


_CACHE = {}


def _get_program(groups, calls, SG, TOT16, bias_zero):
    key = (tuple(groups), tuple(calls), SG, TOT16, bias_zero)
    if key not in _CACHE:
        _CACHE[key] = _build_program(groups, calls, SG, TOT16, bias_zero)
    return _CACHE[key]


def run(x, edge_index, W1, b1, W2, b2, trace=False):
    prep = _host_prep(x, edge_index)
    bias_zero = not (np.any(np.asarray(b1)) or np.any(np.asarray(b2)))
    nc = _get_program(prep["groups"], prep["calls"], prep["SG"], prep["TOT16"],
                      bias_zero)

    W1 = np.ascontiguousarray(np.asarray(W1, np.float32))
    W2 = np.ascontiguousarray(np.asarray(W2, np.float32))
    b1 = np.ascontiguousarray(np.asarray(b1, np.float32))
    b2 = np.ascontiguousarray(np.asarray(b2, np.float32))

    in_maps = []
    for c in range(M):
        in_maps.append({
            "x_pad": prep["x_pad"],
            "deg_arr": prep["deg_arr"],
            "deg_shard": np.ascontiguousarray(prep["deg_shard"][c]),
            "x_shard": np.ascontiguousarray(prep["x_shard"][c].reshape(P, TILES * C)),
            "idx16": np.ascontiguousarray(prep["idx16"][c]),
            "mask": np.ascontiguousarray(prep["mask"][c]),
            "W1": W1, "W2": W2, "b1": b1, "b2": b2,
        })

    res = run_bass_kernel_spmd(nc, in_maps, core_ids=list(range(M)), trace=trace)

    outs = np.stack(
        [res.results[c]["out"].reshape(P, TILES, C) for c in range(M)]
    )  # [M, P, TILES, C]
    t_of = prep["pos_of"] >> 7
    p_of = prep["pos_of"] & 127
    full = outs[prep["core_of"], p_of, t_of]
    return np.ascontiguousarray(full, dtype=np.float32), res


def kernel(x, edge_index, W1, b1, W2, b2):
    out, _ = run(x, edge_index, W1, b1, W2, b2, trace=False)
    return out


# revision 8
# speedup vs baseline: 1.2011x; 1.2011x over previous
"""Trainium2 Bass kernel for nn_DiffusionGraphConv_89936615178296.

out = relu(A_hat @ (x @ (W1+W2)) + b1 + b2), A_hat = D^-1/2 (A+I) D^-1/2.

Reformulation: out = relu(dinv * (Agg(dinv*x) @ W) + b), W = W1+W2,
dinv = rsqrt(1 + in_degree); the per-edge norm dinv[src]*dinv[dst] is
separable so per-edge work is a pure gather + segment-sum of x' = dinv*x.

8 cores, no collectives. Nodes sharded round-robin in degree-sorted order.
Each core:
  P0: builds the fp16 table x' = dinv*x for the full graph in its local HBM.
  P1: per destination tile group, one batched dma_gather (InstDMAGatherAnt,
      int16 element indices = src>>2, each 256B element = 4 packed fp16 rows)
      pulls all slots into a [128, cols, 128] grid; a host-built one-hot
      phase mask (src&3) zeroes the 3 unwanted rows per slot; one 4D-AP
      tensor_reduce folds (slot, phase) per tile; dinv scale; batched
      transpose + block-diag matmul with W on the tensor engine; relu;
      final DMA writes the output shard.

Host does integer index prep only (degree counts, shard permutation, slot
layout, int16 gather indices, 0/1 phase masks); all f32 arithmetic runs on
device.
"""

import numpy as np
from contextlib import ExitStack

import concourse.bass as bass
import concourse.bacc as bacc
import concourse.tile as tile
from concourse import mybir
from concourse.masks import make_identity
from concourse.bass_utils import run_bass_kernel_spmd

N, E, C = 100000, 1600000, 32
M = 8
P = 128
TILES = 98                  # 98*128 = 12544 rows per core >= 12500
TABLE_STRIPS = 783          # 783*128 = 100224 >= N+4 (zero rows 100000..100003)
TABLE_ROWS = TABLE_STRIPS * P
ZERO_ELEM = 25000           # element covering rows 100000..100003 (all zero)
NELEM = TABLE_ROWS * C // 128  # gatherable 256B elements in the table
G_STRIP = 64                # table build strip width
NT = 4                      # tiles per group
CALL_COLS = 120             # max grid columns per dma_gather call (15360 idxs)


def _host_prep(x, edge_index):
    src = np.asarray(edge_index[0], dtype=np.int64)
    dst = np.asarray(edge_index[1], dtype=np.int64)
    x = np.asarray(x, dtype=np.float32)

    deg = np.bincount(dst, minlength=N).astype(np.int64) + 1

    order = np.argsort(-deg, kind="stable")
    ar = np.arange(N)
    core_of = np.empty(N, np.int64)
    pos_of = np.empty(N, np.int64)
    core_of[order] = ar % M
    pos_of[order] = ar // M

    degs_sorted = deg[order]
    # per-tile edge-slot count (self-loop handled via x_shard), >= 1
    Kt = [int(degs_sorted[t * P * M]) - 1 if t * P * M < N else 1 for t in range(TILES)]
    Kt = [max(1, k) for k in Kt]

    # groups of NT tiles with uniform K = max tile K in group
    groups = []  # (g0, g1, KG, colbase)
    col = 0
    for g0 in range(0, TILES, NT):
        g1 = min(g0 + NT, TILES)
        KG = max(Kt[g0:g1])
        groups.append((g0, g1, KG, col))
        col += KG * (g1 - g0)
    SG = col  # total grid columns per core

    # gather calls: split groups at CALL_COLS boundaries
    # call = (col_start, ncols, o16) ; o16 = offset into idx16 array cols
    calls = []
    o16 = 0
    for (g0, g1, KG, cb) in groups:
        ncols_g = KG * (g1 - g0)
        cs = 0
        while cs < ncols_g:
            cc = min(CALL_COLS, ncols_g - cs)
            calls.append((cb + cs, cc, o16))
            o16 += cc * P // 16
            cs += cc
    TOT16 = o16

    t_of = pos_of >> 7
    p_of = pos_of & 127

    # global column of tile t slot k:  colbase(group) + (t%NT)*KG + k
    gidx = np.arange(TILES) // NT
    KG_of_t = np.array([groups[g][2] for g in gidx])
    colbase_of_t = np.array([groups[g][3] for g in gidx]) + (
        np.arange(TILES) - gidx * NT
    ) * KG_of_t

    # CSR by destination: rank of edge within its dst
    eorder = np.argsort(dst, kind="stable")
    src_sorted = src[eorder]
    dstn = dst[eorder]
    starts = np.concatenate([[0], np.cumsum(deg - 1)])
    rank = np.arange(E, dtype=np.int64) - starts[dstn]

    ecore = core_of[dstn]
    gcol = colbase_of_t[t_of[dstn]] + rank
    flat = gcol * P + p_of[dstn]            # slot id within core, col-major

    # int16 element indices (src>>2), pad = ZERO_ELEM
    A = np.full((M, SG * P), ZERO_ELEM, np.int16)
    A[ecore, flat] = (src_sorted >> 2).astype(np.int16)
    # phase mask fp16 [M, P, 4*SG]: 1.0 at (p, 4*gcol + (src&3))
    mask = np.zeros((M, P, 4 * SG), np.float16)
    mask[ecore, p_of[dstn], 4 * gcol + (src_sorted & 3)] = 1.0

    # wrap each call's indices: [16, n/16] with [l, s] = arr[s*16+l], tile x8
    idx16 = np.zeros((M, P, TOT16), np.int16)
    for (c0, cc, o16c) in calls:
        n = cc * P
        sl = A[:, c0 * P : c0 * P + n]                       # [M, n]
        wr = sl.reshape(M, n // 16, 16).transpose(0, 2, 1)   # [M, 16, n/16]
        idx16[:, :, o16c : o16c + n // 16] = np.tile(wr, (1, 8, 1))

    # per-core own-shard features, [P, TILES, C], zero on pad rows
    x_shard = np.zeros((M, P, TILES, C), np.float32)
    x_shard[core_of, p_of, t_of] = x

    deg_pad = np.ones(TABLE_ROWS, np.float32)
    deg_pad[:N] = deg
    deg_arr = np.ones((P, TABLE_STRIPS), np.float32)
    colpos = 0
    for s0 in range(0, TABLE_STRIPS, G_STRIP):
        g = min(G_STRIP, TABLE_STRIPS - s0)
        blk = deg_pad[s0 * P : (s0 + g) * P].reshape(P, g)
        deg_arr[:, colpos : colpos + g] = blk
        colpos += g

    deg_shard = np.ones((M, P, TILES), np.float32)
    deg_shard[core_of, p_of, t_of] = deg.astype(np.float32)

    x_pad = np.zeros((TABLE_ROWS, C), np.float32)
    x_pad[:N] = x

    return dict(
        groups=groups, calls=calls, SG=SG, TOT16=TOT16, idx16=idx16,
        mask=mask, deg_arr=deg_arr, deg_shard=deg_shard, x_pad=x_pad,
        core_of=core_of, pos_of=pos_of, x_shard=x_shard,
    )


def _build_program(groups, calls, SG, TOT16, bias_zero):
    nc = bacc.Bacc("TRN2", target_bir_lowering=False, debug=False, num_devices=M)
    f32, f16, i16 = mybir.dt.float32, mybir.dt.float16, mybir.dt.int16

    x_d = nc.dram_tensor("x_pad", [TABLE_ROWS, C], f32, kind="ExternalInput")
    dega_d = nc.dram_tensor("deg_arr", [P, TABLE_STRIPS], f32, kind="ExternalInput")
    degs_d = nc.dram_tensor("deg_shard", [P, TILES], f32, kind="ExternalInput")
    idx_d = nc.dram_tensor("idx16", [P, TOT16], i16, kind="ExternalInput")
    mask_d = nc.dram_tensor("mask", [P, 4 * SG], f16, kind="ExternalInput")
    xs_d = nc.dram_tensor("x_shard", [P, TILES * C], f32, kind="ExternalInput")
    w1_d = nc.dram_tensor("W1", [C, C], f32, kind="ExternalInput")
    w2_d = nc.dram_tensor("W2", [C, C], f32, kind="ExternalInput")
    b1_d = nc.dram_tensor("b1", [C], f32, kind="ExternalInput")
    b2_d = nc.dram_tensor("b2", [C], f32, kind="ExternalInput")
    table_d = nc.dram_tensor("table", [TABLE_ROWS, C], f16, kind="Internal")
    out_d = nc.dram_tensor("out", [P, TILES * C], f32, kind="ExternalOutput")

    with tile.TileContext(nc) as tc, ExitStack() as ctx:
        singles = ctx.enter_context(tc.tile_pool(name="singles", bufs=1))
        xpool = ctx.enter_context(tc.tile_pool(name="xin", bufs=3))
        tpool = ctx.enter_context(tc.tile_pool(name="tout", bufs=3))
        gpool = ctx.enter_context(tc.tile_pool(name="gather", bufs=2))
        apool = ctx.enter_context(tc.tile_pool(name="agg", bufs=3))
        tspool = ctx.enter_context(tc.tile_pool(name="trsb", bufs=3))
        pst = ctx.enter_context(tc.tile_pool(name="pst", bufs=3, space="PSUM"))
        psm = ctx.enter_context(tc.tile_pool(name="psm", bufs=3, space="PSUM"))

        # ---- singles ----
        w4a = singles.tile([P, P], f32)
        w4b = singles.tile([P, P], f32)
        nc.vector.memset(w4a[:], 0.0)
        nc.gpsimd.memset(w4b[:], 0.0)
        for t in range(NT):
            sl = slice(t * C, (t + 1) * C)
            nc.sync.dma_start(out=w4a[sl, sl], in_=w1_d.ap())
            nc.sync.dma_start(out=w4b[sl, sl], in_=w2_d.ap())
        nc.vector.tensor_add(out=w4a[:], in0=w4a[:], in1=w4b[:])

        if not bias_zero:
            b1_sb = singles.tile([P, C], f32)
            b2_sb = singles.tile([P, C], f32)
            nc.sync.dma_start(
                out=b1_sb[:], in_=bass.AP(tensor=b1_d, offset=0, ap=[[0, P], [1, C]])
            )
            nc.sync.dma_start(
                out=b2_sb[:], in_=bass.AP(tensor=b2_d, offset=0, ap=[[0, P], [1, C]])
            )
            bsum_sb = singles.tile([P, C], f32)
            nc.vector.tensor_add(out=bsum_sb[:], in0=b1_sb[:], in1=b2_sb[:])

        ident = singles.tile([P, P], f32)
        make_identity(nc, ident[:])

        idx_sb = singles.tile([P, TOT16], i16)
        nc.sync.dma_start(out=idx_sb[:], in_=idx_d.ap())
        mask_sb = singles.tile([P, 4 * SG], f16)
        nc.sync.dma_start(out=mask_sb[:], in_=mask_d.ap())

        dinv_all = singles.tile([P, TABLE_STRIPS], f32)
        nc.sync.dma_start(out=dinv_all[:], in_=dega_d.ap())
        nc.scalar.sqrt(out=dinv_all[:], in_=dinv_all[:])
        nc.vector.reciprocal(out=dinv_all[:], in_=dinv_all[:])

        dinv_sh = singles.tile([P, TILES], f32)
        nc.sync.dma_start(out=dinv_sh[:], in_=degs_d.ap())
        nc.scalar.sqrt(out=dinv_sh[:], in_=dinv_sh[:])
        nc.vector.reciprocal(out=dinv_sh[:], in_=dinv_sh[:])

        out_all = singles.tile([P, TILES * C], f32)

        # self-loop contribution x' = dinv * x for own shard rows
        xs_all = singles.tile([P, TILES * C], f32)
        nc.sync.dma_start(out=xs_all[:], in_=xs_d.ap())
        nc.vector.tensor_tensor(
            out=xs_all[:].rearrange("p (t c) -> p t c", c=C),
            in0=xs_all[:].rearrange("p (t c) -> p t c", c=C),
            in1=dinv_sh[:].to_broadcast([P, TILES, C]),
            op=mybir.AluOpType.mult,
        )

        # ---- P0: table build, partition-contiguous strips ----
        colpos = 0
        for s0 in range(0, TABLE_STRIPS, G_STRIP):
            g = min(G_STRIP, TABLE_STRIPS - s0)
            x_ap = bass.AP(
                tensor=x_d, offset=s0 * P * C,
                ap=[[g * C, P], [C, g], [1, C]],
            )
            t_ap = bass.AP(
                tensor=table_d, offset=s0 * P * C,
                ap=[[g * C, P], [C, g], [1, C]],
            )
            x_sb = xpool.tile([P, G_STRIP, C], f32, tag="x")
            nc.sync.dma_start(out=x_sb[:, :g, :], in_=x_ap)
            t_sb = tpool.tile([P, G_STRIP, C], f16, tag="t")
            nc.vector.tensor_tensor(
                out=t_sb[:, :g, :],
                in0=x_sb[:, :g, :],
                in1=dinv_all[:, colpos : colpos + g].to_broadcast([P, g, C]),
                op=mybir.AluOpType.mult,
            )
            nc.sync.dma_start(out=t_ap, in_=t_sb[:, :g, :])
            colpos += g

        # gather source: 256B elements = 4 packed fp16 rows
        table_elems = bass.AP(tensor=table_d, offset=0, ap=[[128, NELEM], [1, 128]])

        # ---- P1 ----
        maxcols = max(KG * (g1 - g0) for g0, g1, KG, _ in groups)
        call_i = 0
        for (g0, g1, KG, cb) in groups:
            nt = g1 - g0
            ncols = KG * nt
            gbuf = gpool.tile([P, maxcols * 128], f16, tag="gbuf")
            # one (or few) batched gathers for the whole group
            cdone = 0
            while cdone < ncols:
                c0, cc, o16c = calls[call_i]
                assert c0 == cb + cdone, (c0, cb, cdone)
                nc.gpsimd.dma_gather(
                    out_ap=gbuf[:, cdone * 128 : (cdone + cc) * 128].rearrange(
                        "p (g e) -> p g e", e=128
                    ),
                    in_ap=table_elems,
                    idxs_ap=idx_sb[:, o16c : o16c + cc * P // 16],
                    num_idxs=cc * P,
                    num_idxs_reg=cc * P,
                    elem_size=128,
                    elem_step=128,
                    single_packet=False,
                )
                call_i += 1
                cdone += cc
            # phase select: grid *= one-hot mask over the 4 rows per element
            nc.vector.tensor_tensor(
                out=gbuf[:, : ncols * 128].rearrange(
                    "p (s m c) -> p s m c", m=4, c=C
                ),
                in0=gbuf[:, : ncols * 128].rearrange(
                    "p (s m c) -> p s m c", m=4, c=C
                ),
                in1=mask_sb[:, 4 * cb : 4 * (cb + ncols)]
                .rearrange("p (s m) -> p s m", m=4)
                .to_broadcast([P, ncols, 4, C]),
                op=mybir.AluOpType.mult,
            )
            # fold (slot, phase) per tile by contiguous pairwise halving,
            # in place in gbuf: view as [p, t, W, C]; each level writes
            # [t, i] from [t, 2i]+[t, 2i+1] (writes trail reads), W -> W/2.
            agg = apool.tile([P, NT * C], f32, tag="agg")
            curw = 4 * KG
            while curw > 1:
                srcv = gbuf[:, : nt * curw * C].rearrange(
                    "p (t w c) -> p t w c", t=nt, c=C
                )
                if curw & 1:
                    # fold the odd leftover into column 0 (write trails reads)
                    nc.vector.tensor_tensor(
                        out=srcv[:, :, 0, :],
                        in0=srcv[:, :, 0, :],
                        in1=srcv[:, :, curw - 1, :],
                        op=mybir.AluOpType.add,
                    )
                    curw -= 1
                H = curw // 2
                dstv = gbuf[:, : nt * H * C].rearrange(
                    "p (t w c) -> p t w c", t=nt, c=C
                )
                nc.vector.tensor_tensor(
                    out=dstv[:, :, :H, :],
                    in0=srcv[:, :, 0 : 2 * H : 2, :],
                    in1=srcv[:, :, 1 : 2 * H : 2, :],
                    op=mybir.AluOpType.add,
                )
                curw = H
            nc.vector.tensor_copy(out=agg[:, : nt * C], in_=gbuf[:, : nt * C])
            # add self-loop term
            nc.vector.tensor_add(
                out=agg[:, : nt * C],
                in0=agg[:, : nt * C],
                in1=xs_all[:, g0 * C : g1 * C],
            )
            # dinv scale (per node row)
            nc.vector.tensor_tensor(
                out=agg[:, : nt * C].rearrange("p (t c) -> p t c", c=C),
                in0=agg[:, : nt * C].rearrange("p (t c) -> p t c", c=C),
                in1=dinv_sh[:, g0:g1].to_broadcast([P, nt, C]),
                op=mybir.AluOpType.mult,
            )
            # batched transpose + block-diag matmul
            trps = pst.tile([P, P], f32, tag="trps")
            nc.tensor.transpose(
                out=trps[: nt * C, :], in_=agg[:, : nt * C], identity=ident[:]
            )
            aggdT = tspool.tile([P, P], f32, tag="aggdT")
            nc.scalar.copy(out=aggdT[: nt * C, :], in_=trps[: nt * C, :])
            mm = psm.tile([P, P], f32, tag="mm")
            nc.tensor.matmul(
                out=mm[:, : nt * C],
                lhsT=aggdT[: nt * C, :],
                rhs=w4a[: nt * C, : nt * C],
                start=True, stop=True,
            )
            osl = out_all[:, g0 * C : g1 * C]
            if bias_zero:
                nc.scalar.activation(
                    out=osl, in_=mm[:, : nt * C],
                    func=mybir.ActivationFunctionType.Relu,
                )
            else:
                for ti in range(nt):
                    nc.vector.tensor_add(
                        out=osl[:, ti * C : (ti + 1) * C],
                        in0=mm[:, ti * C : (ti + 1) * C],
                        in1=bsum_sb[:],
                    )
                nc.scalar.activation(
                    out=osl, in_=osl, func=mybir.ActivationFunctionType.Relu
                )

        nc.sync.dma_start(out=out_d.ap(), in_=out_all[:])

    nc.compile()
    return nc


_CACHE = {}


def _get_program(groups, calls, SG, TOT16, bias_zero):
    key = (tuple(groups), tuple(calls), SG, TOT16, bias_zero)
    if key not in _CACHE:
        _CACHE[key] = _build_program(groups, calls, SG, TOT16, bias_zero)
    return _CACHE[key]


def run(x, edge_index, W1, b1, W2, b2, trace=False):
    prep = _host_prep(x, edge_index)
    bias_zero = not (np.any(np.asarray(b1)) or np.any(np.asarray(b2)))
    nc = _get_program(prep["groups"], prep["calls"], prep["SG"], prep["TOT16"],
                      bias_zero)

    W1 = np.ascontiguousarray(np.asarray(W1, np.float32))
    W2 = np.ascontiguousarray(np.asarray(W2, np.float32))
    b1 = np.ascontiguousarray(np.asarray(b1, np.float32))
    b2 = np.ascontiguousarray(np.asarray(b2, np.float32))

    in_maps = []
    for c in range(M):
        in_maps.append({
            "x_pad": prep["x_pad"],
            "deg_arr": prep["deg_arr"],
            "deg_shard": np.ascontiguousarray(prep["deg_shard"][c]),
            "x_shard": np.ascontiguousarray(prep["x_shard"][c].reshape(P, TILES * C)),
            "idx16": np.ascontiguousarray(prep["idx16"][c]),
            "mask": np.ascontiguousarray(prep["mask"][c]),
            "W1": W1, "W2": W2, "b1": b1, "b2": b2,
        })

    res = run_bass_kernel_spmd(nc, in_maps, core_ids=list(range(M)), trace=trace)

    outs = np.stack(
        [res.results[c]["out"].reshape(P, TILES, C) for c in range(M)]
    )  # [M, P, TILES, C]
    t_of = prep["pos_of"] >> 7
    p_of = prep["pos_of"] & 127
    full = outs[prep["core_of"], p_of, t_of]
    return np.ascontiguousarray(full, dtype=np.float32), res


def kernel(x, edge_index, W1, b1, W2, b2):
    out, _ = run(x, edge_index, W1, b1, W2, b2, trace=False)
    return out


# revision 10
# speedup vs baseline: 1.3417x; 1.1171x over previous
"""Trainium2 Bass kernel for nn_DiffusionGraphConv_89936615178296.

out = relu(A_hat @ (x @ (W1+W2)) + b1 + b2), A_hat = D^-1/2 (A+I) D^-1/2.

Reformulation: out = relu(dinv * (Agg(dinv*x) @ W) + b), W = W1+W2,
dinv = rsqrt(1 + in_degree); the per-edge norm dinv[src]*dinv[dst] is
separable so per-edge work is a pure gather + segment-sum of x' = dinv*x.

8 cores, no collectives. Nodes sharded round-robin in degree-sorted order.
Each core:
  P0: builds the fp16 table x' = dinv*x for the full graph in its local HBM.
  P1: per destination tile group, one batched dma_gather (InstDMAGatherAnt,
      int16 element indices = src>>2, each 256B element = 4 packed fp16 rows)
      pulls all slots into a [128, cols, 128] grid; a host-built one-hot
      phase mask (src&3) zeroes the 3 unwanted rows per slot; one 4D-AP
      tensor_reduce folds (slot, phase) per tile; dinv scale; batched
      transpose + block-diag matmul with W on the tensor engine; relu;
      final DMA writes the output shard.

Host does integer index prep only (degree counts, shard permutation, slot
layout, int16 gather indices, 0/1 phase masks); all f32 arithmetic runs on
device.
"""

import numpy as np
from contextlib import ExitStack

import concourse.bass as bass
import concourse.bacc as bacc
import concourse.tile as tile
from concourse import mybir
from concourse.masks import make_identity
from concourse.bass_utils import run_bass_kernel_spmd

N, E, C = 100000, 1600000, 32
M = 8
P = 128
TILES = 98                  # 98*128 = 12544 rows per core >= 12500
TABLE_STRIPS = 783          # 783*128 = 100224 >= N+4 (zero rows 100000..100003)
TABLE_ROWS = TABLE_STRIPS * P
ZERO_ELEM = 25000           # element covering rows 100000..100003 (all zero)
NELEM = TABLE_ROWS * C // 128  # gatherable 256B elements in the table
G_STRIP = 64                # table build strip width
NT = 4                      # tiles per group
CALL_COLS = 120             # max grid columns per dma_gather call (15360 idxs)
GROUP_COLS = 96             # max grid columns per group (grid tile = 24KB/partition)


def _host_prep(x, edge_index):
    src = np.asarray(edge_index[0], dtype=np.int64)
    dst = np.asarray(edge_index[1], dtype=np.int64)
    x = np.asarray(x, dtype=np.float32)

    deg = np.bincount(dst, minlength=N).astype(np.int64) + 1

    order = np.argsort(-deg, kind="stable")
    ar = np.arange(N)
    core_of = np.empty(N, np.int64)
    pos_of = np.empty(N, np.int64)
    core_of[order] = ar % M
    pos_of[order] = ar // M

    degs_sorted = deg[order]
    # per-tile edge-slot count (self-loop handled via x_shard), >= 1
    Kt = [int(degs_sorted[t * P * M]) - 1 if t * P * M < N else 1 for t in range(TILES)]
    Kt = [max(1, k) for k in Kt]

    # groups of up to NT tiles with uniform K = max tile K in group;
    # cap group width (KG * nt) at GROUP_COLS so 3 grid buffers fit in SBUF
    groups = []  # (g0, g1, KG, colbase)
    col = 0
    g0 = 0
    while g0 < TILES:
        KG = max(1, Kt[g0])
        nt = max(1, min(NT, GROUP_COLS // KG))
        g1 = min(g0 + nt, TILES)
        KG = max(Kt[g0:g1])
        groups.append((g0, g1, KG, col))
        col += KG * (g1 - g0)
        g0 = g1
    SG = col  # total grid columns per core

    # gather calls: split groups at CALL_COLS boundaries
    # call = (col_start, ncols, o16) ; o16 = offset into idx16 array cols
    calls = []
    o16 = 0
    for (g0, g1, KG, cb) in groups:
        ncols_g = KG * (g1 - g0)
        cs = 0
        while cs < ncols_g:
            cc = min(CALL_COLS, ncols_g - cs)
            calls.append((cb + cs, cc, o16))
            o16 += cc * P // 16
            cs += cc
    TOT16 = o16

    t_of = pos_of >> 7
    p_of = pos_of & 127

    # global column of tile t slot k:  colbase(group) + (t-g0)*KG + k
    KG_of_t = np.empty(TILES, np.int64)
    colbase_of_t = np.empty(TILES, np.int64)
    for (g0, g1, KG, cb) in groups:
        for t in range(g0, g1):
            KG_of_t[t] = KG
            colbase_of_t[t] = cb + (t - g0) * KG

    # CSR by destination: rank of edge within its dst
    eorder = np.argsort(dst, kind="stable")
    src_sorted = src[eorder]
    dstn = dst[eorder]
    starts = np.concatenate([[0], np.cumsum(deg - 1)])
    rank = np.arange(E, dtype=np.int64) - starts[dstn]

    ecore = core_of[dstn]
    gcol = colbase_of_t[t_of[dstn]] + rank
    flat = gcol * P + p_of[dstn]            # slot id within core, col-major

    # int16 element indices (src>>2), pad = ZERO_ELEM
    A = np.full((M, SG * P), ZERO_ELEM, np.int16)
    A[ecore, flat] = (src_sorted >> 2).astype(np.int16)
    # phase mask fp16 [M, P, 4*SG]: 1.0 at (p, 4*gcol + (src&3))
    mask = np.zeros((M, P, 4 * SG), np.float16)
    mask[ecore, p_of[dstn], 4 * gcol + (src_sorted & 3)] = 1.0

    # wrap each call's indices: [16, n/16] with [l, s] = arr[s*16+l], tile x8
    idx16 = np.zeros((M, P, TOT16), np.int16)
    for (c0, cc, o16c) in calls:
        n = cc * P
        sl = A[:, c0 * P : c0 * P + n]                       # [M, n]
        wr = sl.reshape(M, n // 16, 16).transpose(0, 2, 1)   # [M, 16, n/16]
        idx16[:, :, o16c : o16c + n // 16] = np.tile(wr, (1, 8, 1))

    # per-core own-shard features, [P, TILES, C], zero on pad rows
    x_shard = np.zeros((M, P, TILES, C), np.float32)
    x_shard[core_of, p_of, t_of] = x

    deg_pad = np.ones(TABLE_ROWS, np.float32)
    deg_pad[:N] = deg
    deg_arr = np.ones((P, TABLE_STRIPS), np.float32)
    colpos = 0
    for s0 in range(0, TABLE_STRIPS, G_STRIP):
        g = min(G_STRIP, TABLE_STRIPS - s0)
        blk = deg_pad[s0 * P : (s0 + g) * P].reshape(P, g)
        deg_arr[:, colpos : colpos + g] = blk
        colpos += g

    deg_shard = np.ones((M, P, TILES), np.float32)
    deg_shard[core_of, p_of, t_of] = deg.astype(np.float32)

    x_pad = np.zeros((TABLE_ROWS, C), np.float32)
    x_pad[:N] = x

    return dict(
        groups=groups, calls=calls, SG=SG, TOT16=TOT16, idx16=idx16,
        mask=mask, deg_arr=deg_arr, deg_shard=deg_shard, x_pad=x_pad,
        core_of=core_of, pos_of=pos_of, x_shard=x_shard,
    )


def _build_program(groups, calls, SG, TOT16, bias_zero):
    nc = bacc.Bacc("TRN2", target_bir_lowering=False, debug=False, num_devices=M)
    f32, f16, i16 = mybir.dt.float32, mybir.dt.float16, mybir.dt.int16

    x_d = nc.dram_tensor("x_pad", [TABLE_ROWS, C], f32, kind="ExternalInput")
    dega_d = nc.dram_tensor("deg_arr", [P, TABLE_STRIPS], f32, kind="ExternalInput")
    degs_d = nc.dram_tensor("deg_shard", [P, TILES], f32, kind="ExternalInput")
    idx_d = nc.dram_tensor("idx16", [P, TOT16], i16, kind="ExternalInput")
    mask_d = nc.dram_tensor("mask", [P, 4 * SG], f16, kind="ExternalInput")
    xs_d = nc.dram_tensor("x_shard", [P, TILES * C], f32, kind="ExternalInput")
    w1_d = nc.dram_tensor("W1", [C, C], f32, kind="ExternalInput")
    w2_d = nc.dram_tensor("W2", [C, C], f32, kind="ExternalInput")
    b1_d = nc.dram_tensor("b1", [C], f32, kind="ExternalInput")
    b2_d = nc.dram_tensor("b2", [C], f32, kind="ExternalInput")
    table_d = nc.dram_tensor("table", [TABLE_ROWS, C], f16, kind="Internal")
    out_d = nc.dram_tensor("out", [P, TILES * C], f32, kind="ExternalOutput")

    with tile.TileContext(nc) as tc, ExitStack() as ctx:
        singles = ctx.enter_context(tc.tile_pool(name="singles", bufs=1))
        xpool = ctx.enter_context(tc.tile_pool(name="xin", bufs=3))
        tpool = ctx.enter_context(tc.tile_pool(name="tout", bufs=3))
        gpool = ctx.enter_context(tc.tile_pool(name="gather", bufs=3))
        apool = ctx.enter_context(tc.tile_pool(name="agg", bufs=3))
        tspool = ctx.enter_context(tc.tile_pool(name="trsb", bufs=3))
        pst = ctx.enter_context(tc.tile_pool(name="pst", bufs=3, space="PSUM"))
        psm = ctx.enter_context(tc.tile_pool(name="psm", bufs=3, space="PSUM"))

        # ---- singles ----
        w4a = singles.tile([P, P], f32)
        w4b = singles.tile([P, P], f32)
        nc.vector.memset(w4a[:], 0.0)
        nc.gpsimd.memset(w4b[:], 0.0)
        for t in range(NT):
            sl = slice(t * C, (t + 1) * C)
            nc.sync.dma_start(out=w4a[sl, sl], in_=w1_d.ap())
            nc.sync.dma_start(out=w4b[sl, sl], in_=w2_d.ap())
        nc.vector.tensor_add(out=w4a[:], in0=w4a[:], in1=w4b[:])

        if not bias_zero:
            b1_sb = singles.tile([P, C], f32)
            b2_sb = singles.tile([P, C], f32)
            nc.sync.dma_start(
                out=b1_sb[:], in_=bass.AP(tensor=b1_d, offset=0, ap=[[0, P], [1, C]])
            )
            nc.sync.dma_start(
                out=b2_sb[:], in_=bass.AP(tensor=b2_d, offset=0, ap=[[0, P], [1, C]])
            )
            bsum_sb = singles.tile([P, C], f32)
            nc.vector.tensor_add(out=bsum_sb[:], in0=b1_sb[:], in1=b2_sb[:])

        ident = singles.tile([P, P], f32)
        make_identity(nc, ident[:])

        idx_sb = singles.tile([P, TOT16], i16)
        nc.sync.dma_start(out=idx_sb[:], in_=idx_d.ap())
        mask_sb = singles.tile([P, 4 * SG], f16)
        nc.sync.dma_start(out=mask_sb[:], in_=mask_d.ap())

        dinv_all = singles.tile([P, TABLE_STRIPS], f32)
        nc.sync.dma_start(out=dinv_all[:], in_=dega_d.ap())
        nc.scalar.sqrt(out=dinv_all[:], in_=dinv_all[:])
        nc.vector.reciprocal(out=dinv_all[:], in_=dinv_all[:])

        dinv_sh = singles.tile([P, TILES], f32)
        nc.sync.dma_start(out=dinv_sh[:], in_=degs_d.ap())
        nc.scalar.sqrt(out=dinv_sh[:], in_=dinv_sh[:])
        nc.vector.reciprocal(out=dinv_sh[:], in_=dinv_sh[:])

        out_all = singles.tile([P, TILES * C], f32)

        # self-loop contribution x' = dinv * x for own shard rows
        xs_all = singles.tile([P, TILES * C], f32)
        nc.sync.dma_start(out=xs_all[:], in_=xs_d.ap())
        nc.vector.tensor_tensor(
            out=xs_all[:].rearrange("p (t c) -> p t c", c=C),
            in0=xs_all[:].rearrange("p (t c) -> p t c", c=C),
            in1=dinv_sh[:].to_broadcast([P, TILES, C]),
            op=mybir.AluOpType.mult,
        )

        # ---- P0: table build, partition-contiguous strips ----
        colpos = 0
        for s0 in range(0, TABLE_STRIPS, G_STRIP):
            g = min(G_STRIP, TABLE_STRIPS - s0)
            x_ap = bass.AP(
                tensor=x_d, offset=s0 * P * C,
                ap=[[g * C, P], [C, g], [1, C]],
            )
            t_ap = bass.AP(
                tensor=table_d, offset=s0 * P * C,
                ap=[[g * C, P], [C, g], [1, C]],
            )
            x_sb = xpool.tile([P, G_STRIP, C], f32, tag="x")
            nc.sync.dma_start(out=x_sb[:, :g, :], in_=x_ap)
            t_sb = tpool.tile([P, G_STRIP, C], f16, tag="t")
            nc.vector.tensor_tensor(
                out=t_sb[:, :g, :],
                in0=x_sb[:, :g, :],
                in1=dinv_all[:, colpos : colpos + g].to_broadcast([P, g, C]),
                op=mybir.AluOpType.mult,
            )
            nc.sync.dma_start(out=t_ap, in_=t_sb[:, :g, :])
            colpos += g

        # gather source: 256B elements = 4 packed fp16 rows
        table_elems = bass.AP(tensor=table_d, offset=0, ap=[[128, NELEM], [1, 128]])

        # ---- P1 ----
        maxcols = max(KG * (g1 - g0) for g0, g1, KG, _ in groups)
        call_i = 0
        for (g0, g1, KG, cb) in groups:
            nt = g1 - g0
            ncols = KG * nt
            gbuf = gpool.tile([P, maxcols * 128], f16, tag="gbuf")
            # one (or few) batched gathers for the whole group
            cdone = 0
            while cdone < ncols:
                c0, cc, o16c = calls[call_i]
                assert c0 == cb + cdone, (c0, cb, cdone)
                nc.gpsimd.dma_gather(
                    out_ap=gbuf[:, cdone * 128 : (cdone + cc) * 128].rearrange(
                        "p (g e) -> p g e", e=128
                    ),
                    in_ap=table_elems,
                    idxs_ap=idx_sb[:, o16c : o16c + cc * P // 16],
                    num_idxs=cc * P,
                    num_idxs_reg=cc * P,
                    elem_size=128,
                    elem_step=128,
                    single_packet=False,
                )
                call_i += 1
                cdone += cc
            # phase select: grid *= one-hot mask over the 4 rows per element
            nc.vector.tensor_tensor(
                out=gbuf[:, : ncols * 128].rearrange(
                    "p (s m c) -> p s m c", m=4, c=C
                ),
                in0=gbuf[:, : ncols * 128].rearrange(
                    "p (s m c) -> p s m c", m=4, c=C
                ),
                in1=mask_sb[:, 4 * cb : 4 * (cb + ncols)]
                .rearrange("p (s m) -> p s m", m=4)
                .to_broadcast([P, ncols, 4, C]),
                op=mybir.AluOpType.mult,
            )
            # fold (slot, phase) per tile by contiguous pairwise halving,
            # in place in gbuf: view as [p, t, W, C]; each level writes
            # [t, i] from [t, 2i]+[t, 2i+1] (writes trail reads), W -> W/2.
            agg = apool.tile([P, NT * C], f32, tag="agg")
            curw = 4 * KG
            while curw > 1:
                srcv = gbuf[:, : nt * curw * C].rearrange(
                    "p (t w c) -> p t w c", t=nt, c=C
                )
                if curw & 1:
                    # fold the odd leftover into column 0 (write trails reads)
                    nc.vector.tensor_tensor(
                        out=srcv[:, :, 0, :],
                        in0=srcv[:, :, 0, :],
                        in1=srcv[:, :, curw - 1, :],
                        op=mybir.AluOpType.add,
                    )
                    curw -= 1
                H = curw // 2
                dstv = gbuf[:, : nt * H * C].rearrange(
                    "p (t w c) -> p t w c", t=nt, c=C
                )
                nc.vector.tensor_tensor(
                    out=dstv[:, :, :H, :],
                    in0=srcv[:, :, 0 : 2 * H : 2, :],
                    in1=srcv[:, :, 1 : 2 * H : 2, :],
                    op=mybir.AluOpType.add,
                )
                curw = H
            nc.vector.tensor_copy(out=agg[:, : nt * C], in_=gbuf[:, : nt * C])
            # add self-loop term
            nc.vector.tensor_add(
                out=agg[:, : nt * C],
                in0=agg[:, : nt * C],
                in1=xs_all[:, g0 * C : g1 * C],
            )
            # dinv scale (per node row)
            nc.vector.tensor_tensor(
                out=agg[:, : nt * C].rearrange("p (t c) -> p t c", c=C),
                in0=agg[:, : nt * C].rearrange("p (t c) -> p t c", c=C),
                in1=dinv_sh[:, g0:g1].to_broadcast([P, nt, C]),
                op=mybir.AluOpType.mult,
            )
            # batched transpose + block-diag matmul
            trps = pst.tile([P, P], f32, tag="trps")
            nc.tensor.transpose(
                out=trps[: nt * C, :], in_=agg[:, : nt * C], identity=ident[:]
            )
            aggdT = tspool.tile([P, P], f32, tag="aggdT")
            nc.scalar.copy(out=aggdT[: nt * C, :], in_=trps[: nt * C, :])
            mm = psm.tile([P, P], f32, tag="mm")
            nc.tensor.matmul(
                out=mm[:, : nt * C],
                lhsT=aggdT[: nt * C, :],
                rhs=w4a[: nt * C, : nt * C],
                start=True, stop=True,
            )
            osl = out_all[:, g0 * C : g1 * C]
            if bias_zero:
                nc.scalar.activation(
                    out=osl, in_=mm[:, : nt * C],
                    func=mybir.ActivationFunctionType.Relu,
                )
            else:
                for ti in range(nt):
                    nc.vector.tensor_add(
                        out=osl[:, ti * C : (ti + 1) * C],
                        in0=mm[:, ti * C : (ti + 1) * C],
                        in1=bsum_sb[:],
                    )
                nc.scalar.activation(
                    out=osl, in_=osl, func=mybir.ActivationFunctionType.Relu
                )

        nc.sync.dma_start(out=out_d.ap(), in_=out_all[:])

    nc.compile()
    return nc


_CACHE = {}


def _get_program(groups, calls, SG, TOT16, bias_zero):
    key = (tuple(groups), tuple(calls), SG, TOT16, bias_zero)
    if key not in _CACHE:
        _CACHE[key] = _build_program(groups, calls, SG, TOT16, bias_zero)
    return _CACHE[key]


def run(x, edge_index, W1, b1, W2, b2, trace=False):
    prep = _host_prep(x, edge_index)
    bias_zero = not (np.any(np.asarray(b1)) or np.any(np.asarray(b2)))
    nc = _get_program(prep["groups"], prep["calls"], prep["SG"], prep["TOT16"],
                      bias_zero)

    W1 = np.ascontiguousarray(np.asarray(W1, np.float32))
    W2 = np.ascontiguousarray(np.asarray(W2, np.float32))
    b1 = np.ascontiguousarray(np.asarray(b1, np.float32))
    b2 = np.ascontiguousarray(np.asarray(b2, np.float32))

    in_maps = []
    for c in range(M):
        in_maps.append({
            "x_pad": prep["x_pad"],
            "deg_arr": prep["deg_arr"],
            "deg_shard": np.ascontiguousarray(prep["deg_shard"][c]),
            "x_shard": np.ascontiguousarray(prep["x_shard"][c].reshape(P, TILES * C)),
            "idx16": np.ascontiguousarray(prep["idx16"][c]),
            "mask": np.ascontiguousarray(prep["mask"][c]),
            "W1": W1, "W2": W2, "b1": b1, "b2": b2,
        })

    res = run_bass_kernel_spmd(nc, in_maps, core_ids=list(range(M)), trace=trace)

    outs = np.stack(
        [res.results[c]["out"].reshape(P, TILES, C) for c in range(M)]
    )  # [M, P, TILES, C]
    t_of = prep["pos_of"] >> 7
    p_of = prep["pos_of"] & 127
    full = outs[prep["core_of"], p_of, t_of]
    return np.ascontiguousarray(full, dtype=np.float32), res


def kernel(x, edge_index, W1, b1, W2, b2):
    out, _ = run(x, edge_index, W1, b1, W2, b2, trace=False)
    return out


# revision 11
# speedup vs baseline: 1.3684x; 1.0199x over previous
"""Trainium2 Bass kernel for nn_DiffusionGraphConv_89936615178296.

out = relu(A_hat @ (x @ (W1+W2)) + b1 + b2), A_hat = D^-1/2 (A+I) D^-1/2.

Reformulation: out = relu(dinv * (Agg(dinv*x) @ W) + b), W = W1+W2,
dinv = rsqrt(1 + in_degree); the per-edge norm dinv[src]*dinv[dst] is
separable so per-edge work is a pure gather + segment-sum of x' = dinv*x.

8 cores, no collectives. Nodes sharded round-robin in degree-sorted order.
Each core:
  P0: builds the fp16 table x' = dinv*x for the full graph in its local HBM.
  P1: per destination tile group, one batched dma_gather (InstDMAGatherAnt,
      int16 element indices = src>>2, each 256B element = 4 packed fp16 rows)
      pulls all slots into a [128, cols, 128] grid; a host-built one-hot
      phase mask (src&3) zeroes the 3 unwanted rows per slot; one 4D-AP
      tensor_reduce folds (slot, phase) per tile; dinv scale; batched
      transpose + block-diag matmul with W on the tensor engine; relu;
      final DMA writes the output shard.

Host does integer index prep only (degree counts, shard permutation, slot
layout, int16 gather indices, 0/1 phase masks); all f32 arithmetic runs on
device.
"""

import numpy as np
from contextlib import ExitStack

import concourse.bass as bass
import concourse.bacc as bacc
import concourse.tile as tile
from concourse import mybir
from concourse.masks import make_identity
from concourse.bass_utils import run_bass_kernel_spmd

N, E, C = 100000, 1600000, 32
M = 8
P = 128
TILES = 98                  # 98*128 = 12544 rows per core >= 12500
TABLE_STRIPS = 783          # 783*128 = 100224 >= N+4 (zero rows 100000..100003)
TABLE_ROWS = TABLE_STRIPS * P
ZERO_ELEM = 25000           # element covering rows 100000..100003 (all zero)
NELEM = TABLE_ROWS * C // 128  # gatherable 256B elements in the table
G_STRIP = 64                # table build strip width
NT = 4                      # tiles per group
CALL_COLS = 120             # max grid columns per dma_gather call (15360 idxs)
GROUP_COLS = 96             # max grid columns per group (grid tile = 24KB/partition)


def _host_prep(x, edge_index):
    src = np.asarray(edge_index[0], dtype=np.int64)
    dst = np.asarray(edge_index[1], dtype=np.int64)
    x = np.asarray(x, dtype=np.float32)

    deg = np.bincount(dst, minlength=N).astype(np.int64) + 1

    order = np.argsort(-deg, kind="stable")
    ar = np.arange(N)
    core_of = np.empty(N, np.int64)
    pos_of = np.empty(N, np.int64)
    core_of[order] = ar % M
    pos_of[order] = ar // M

    degs_sorted = deg[order]
    # per-tile edge-slot count (self-loop handled via x_shard), >= 1
    Kt = [int(degs_sorted[t * P * M]) - 1 if t * P * M < N else 1 for t in range(TILES)]
    Kt = [max(1, k) for k in Kt]

    # groups of up to NT tiles with uniform K = max tile K in group;
    # cap group width (KG * nt) at GROUP_COLS so 3 grid buffers fit in SBUF
    groups = []  # (g0, g1, KG, colbase)
    col = 0
    g0 = 0
    while g0 < TILES:
        KG = max(1, Kt[g0])
        nt = max(1, min(NT, GROUP_COLS // KG))
        g1 = min(g0 + nt, TILES)
        KG = max(Kt[g0:g1])
        groups.append((g0, g1, KG, col))
        col += KG * (g1 - g0)
        g0 = g1
    SG = col  # total grid columns per core

    # gather calls: split groups at CALL_COLS boundaries
    # call = (col_start, ncols, o16) ; o16 = offset into idx16 array cols
    calls = []
    o16 = 0
    for (g0, g1, KG, cb) in groups:
        ncols_g = KG * (g1 - g0)
        cs = 0
        while cs < ncols_g:
            cc = min(CALL_COLS, ncols_g - cs)
            calls.append((cb + cs, cc, o16))
            o16 += cc * P // 16
            cs += cc
    TOT16 = o16

    t_of = pos_of >> 7
    p_of = pos_of & 127

    # global column of tile t slot k:  colbase(group) + (t-g0)*KG + k
    KG_of_t = np.empty(TILES, np.int64)
    colbase_of_t = np.empty(TILES, np.int64)
    for (g0, g1, KG, cb) in groups:
        for t in range(g0, g1):
            KG_of_t[t] = KG
            colbase_of_t[t] = cb + (t - g0) * KG

    # CSR by destination: rank of edge within its dst
    eorder = np.argsort(dst, kind="stable")
    src_sorted = src[eorder]
    dstn = dst[eorder]
    starts = np.concatenate([[0], np.cumsum(deg - 1)])
    rank = np.arange(E, dtype=np.int64) - starts[dstn]

    ecore = core_of[dstn]
    gcol = colbase_of_t[t_of[dstn]] + rank
    flat = gcol * P + p_of[dstn]            # slot id within core, col-major

    # int16 element indices (src>>2), pad = ZERO_ELEM
    A = np.full((M, SG * P), ZERO_ELEM, np.int16)
    A[ecore, flat] = (src_sorted >> 2).astype(np.int16)
    # phase mask fp16 [M, P, 4*SG]: 1.0 at (p, 4*gcol + (src&3))
    mask = np.zeros((M, P, 4 * SG), np.float16)
    mask[ecore, p_of[dstn], 4 * gcol + (src_sorted & 3)] = 1.0

    # wrap each call's indices: [16, n/16] with [l, s] = arr[s*16+l], tile x8
    idx16 = np.zeros((M, P, TOT16), np.int16)
    for (c0, cc, o16c) in calls:
        n = cc * P
        sl = A[:, c0 * P : c0 * P + n]                       # [M, n]
        wr = sl.reshape(M, n // 16, 16).transpose(0, 2, 1)   # [M, 16, n/16]
        idx16[:, :, o16c : o16c + n // 16] = np.tile(wr, (1, 8, 1))

    # per-core own-shard features, [P, TILES, C], zero on pad rows
    x_shard = np.zeros((M, P, TILES, C), np.float32)
    x_shard[core_of, p_of, t_of] = x

    deg_pad = np.ones(TABLE_ROWS, np.float32)
    deg_pad[:N] = deg
    deg_arr = np.ones((P, TABLE_STRIPS), np.float32)
    colpos = 0
    for s0 in range(0, TABLE_STRIPS, G_STRIP):
        g = min(G_STRIP, TABLE_STRIPS - s0)
        blk = deg_pad[s0 * P : (s0 + g) * P].reshape(P, g)
        deg_arr[:, colpos : colpos + g] = blk
        colpos += g

    deg_shard = np.ones((M, P, TILES), np.float32)
    deg_shard[core_of, p_of, t_of] = deg.astype(np.float32)

    x_pad = np.zeros((TABLE_ROWS, C), np.float32)
    x_pad[:N] = x

    return dict(
        groups=groups, calls=calls, SG=SG, TOT16=TOT16, idx16=idx16,
        mask=mask, deg_arr=deg_arr, deg_shard=deg_shard, x_pad=x_pad,
        core_of=core_of, pos_of=pos_of, x_shard=x_shard,
    )


def _build_program(groups, calls, SG, TOT16, bias_zero):
    nc = bacc.Bacc("TRN2", target_bir_lowering=False, debug=False, num_devices=M)
    f32, f16, i16 = mybir.dt.float32, mybir.dt.float16, mybir.dt.int16

    x_d = nc.dram_tensor("x_pad", [TABLE_ROWS, C], f32, kind="ExternalInput")
    dega_d = nc.dram_tensor("deg_arr", [P, TABLE_STRIPS], f32, kind="ExternalInput")
    degs_d = nc.dram_tensor("deg_shard", [P, TILES], f32, kind="ExternalInput")
    idx_d = nc.dram_tensor("idx16", [P, TOT16], i16, kind="ExternalInput")
    mask_d = nc.dram_tensor("mask", [P, 4 * SG], f16, kind="ExternalInput")
    xs_d = nc.dram_tensor("x_shard", [P, TILES * C], f32, kind="ExternalInput")
    w1_d = nc.dram_tensor("W1", [C, C], f32, kind="ExternalInput")
    w2_d = nc.dram_tensor("W2", [C, C], f32, kind="ExternalInput")
    b1_d = nc.dram_tensor("b1", [C], f32, kind="ExternalInput")
    b2_d = nc.dram_tensor("b2", [C], f32, kind="ExternalInput")
    table_d = nc.dram_tensor("table", [TABLE_ROWS, C], f16, kind="Internal")
    out_d = nc.dram_tensor("out", [P, TILES * C], f32, kind="ExternalOutput")

    with tile.TileContext(nc) as tc, ExitStack() as ctx:
        singles = ctx.enter_context(tc.tile_pool(name="singles", bufs=1))
        xpool = ctx.enter_context(tc.tile_pool(name="xin", bufs=3))
        tpool = ctx.enter_context(tc.tile_pool(name="tout", bufs=3))
        gpool = ctx.enter_context(tc.tile_pool(name="gather", bufs=4))
        apool = ctx.enter_context(tc.tile_pool(name="agg", bufs=3))
        tspool = ctx.enter_context(tc.tile_pool(name="trsb", bufs=3))
        pst = ctx.enter_context(tc.tile_pool(name="pst", bufs=3, space="PSUM"))
        psm = ctx.enter_context(tc.tile_pool(name="psm", bufs=3, space="PSUM"))

        # ---- singles ----
        w4a = singles.tile([P, P], f32)
        w4b = singles.tile([P, P], f32)
        nc.vector.memset(w4a[:], 0.0)
        nc.gpsimd.memset(w4b[:], 0.0)
        for t in range(NT):
            sl = slice(t * C, (t + 1) * C)
            nc.sync.dma_start(out=w4a[sl, sl], in_=w1_d.ap())
            nc.sync.dma_start(out=w4b[sl, sl], in_=w2_d.ap())
        nc.vector.tensor_add(out=w4a[:], in0=w4a[:], in1=w4b[:])

        if not bias_zero:
            b1_sb = singles.tile([P, C], f32)
            b2_sb = singles.tile([P, C], f32)
            nc.sync.dma_start(
                out=b1_sb[:], in_=bass.AP(tensor=b1_d, offset=0, ap=[[0, P], [1, C]])
            )
            nc.sync.dma_start(
                out=b2_sb[:], in_=bass.AP(tensor=b2_d, offset=0, ap=[[0, P], [1, C]])
            )
            bsum_sb = singles.tile([P, C], f32)
            nc.vector.tensor_add(out=bsum_sb[:], in0=b1_sb[:], in1=b2_sb[:])

        ident = singles.tile([P, P], f32)
        make_identity(nc, ident[:])

        dinv_all = singles.tile([P, TABLE_STRIPS], f32)
        nc.sync.dma_start(out=dinv_all[:], in_=dega_d.ap())
        nc.scalar.sqrt(out=dinv_all[:], in_=dinv_all[:])
        nc.vector.reciprocal(out=dinv_all[:], in_=dinv_all[:])

        dinv_sh = singles.tile([P, TILES], f32)
        nc.sync.dma_start(out=dinv_sh[:], in_=degs_d.ap())
        nc.scalar.sqrt(out=dinv_sh[:], in_=dinv_sh[:])
        nc.vector.reciprocal(out=dinv_sh[:], in_=dinv_sh[:])

        out_all = singles.tile([P, TILES * C], f32)

        # ---- P0: table build, partition-contiguous strips ----
        colpos = 0
        for s0 in range(0, TABLE_STRIPS, G_STRIP):
            g = min(G_STRIP, TABLE_STRIPS - s0)
            x_ap = bass.AP(
                tensor=x_d, offset=s0 * P * C,
                ap=[[g * C, P], [C, g], [1, C]],
            )
            t_ap = bass.AP(
                tensor=table_d, offset=s0 * P * C,
                ap=[[g * C, P], [C, g], [1, C]],
            )
            x_sb = xpool.tile([P, G_STRIP, C], f32, tag="x")
            nc.sync.dma_start(out=x_sb[:, :g, :], in_=x_ap)
            t_sb = tpool.tile([P, G_STRIP, C], f16, tag="t")
            nc.vector.tensor_tensor(
                out=t_sb[:, :g, :],
                in0=x_sb[:, :g, :],
                in1=dinv_all[:, colpos : colpos + g].to_broadcast([P, g, C]),
                op=mybir.AluOpType.mult,
            )
            nc.sync.dma_start(out=t_ap, in_=t_sb[:, :g, :])
            colpos += g

        # deferred singles: loaded after the table strips so P0 gets
        # full DMA bandwidth; all are ready well before their consumers
        idx_sb = singles.tile([P, TOT16], i16)
        nc.sync.dma_start(out=idx_sb[:], in_=idx_d.ap())
        mask_sb = singles.tile([P, 4 * SG], f16)
        nc.sync.dma_start(out=mask_sb[:], in_=mask_d.ap())
        xs_all = singles.tile([P, TILES * C], f32)
        nc.sync.dma_start(out=xs_all[:], in_=xs_d.ap())
        nc.vector.tensor_tensor(
            out=xs_all[:].rearrange("p (t c) -> p t c", c=C),
            in0=xs_all[:].rearrange("p (t c) -> p t c", c=C),
            in1=dinv_sh[:].to_broadcast([P, TILES, C]),
            op=mybir.AluOpType.mult,
        )

        # gather source: 256B elements = 4 packed fp16 rows
        table_elems = bass.AP(tensor=table_d, offset=0, ap=[[128, NELEM], [1, 128]])

        # ---- P1 ----
        maxcols = max(KG * (g1 - g0) for g0, g1, KG, _ in groups)
        call_i = 0
        for (g0, g1, KG, cb) in groups:
            nt = g1 - g0
            ncols = KG * nt
            gbuf = gpool.tile([P, maxcols * 128], f16, tag="gbuf")
            # one (or few) batched gathers for the whole group
            cdone = 0
            while cdone < ncols:
                c0, cc, o16c = calls[call_i]
                assert c0 == cb + cdone, (c0, cb, cdone)
                nc.gpsimd.dma_gather(
                    out_ap=gbuf[:, cdone * 128 : (cdone + cc) * 128].rearrange(
                        "p (g e) -> p g e", e=128
                    ),
                    in_ap=table_elems,
                    idxs_ap=idx_sb[:, o16c : o16c + cc * P // 16],
                    num_idxs=cc * P,
                    num_idxs_reg=cc * P,
                    elem_size=128,
                    elem_step=128,
                    single_packet=False,
                )
                call_i += 1
                cdone += cc
            # phase select: grid *= one-hot mask over the 4 rows per element
            nc.vector.tensor_tensor(
                out=gbuf[:, : ncols * 128].rearrange(
                    "p (s m c) -> p s m c", m=4, c=C
                ),
                in0=gbuf[:, : ncols * 128].rearrange(
                    "p (s m c) -> p s m c", m=4, c=C
                ),
                in1=mask_sb[:, 4 * cb : 4 * (cb + ncols)]
                .rearrange("p (s m) -> p s m", m=4)
                .to_broadcast([P, ncols, 4, C]),
                op=mybir.AluOpType.mult,
            )
            # fold (slot, phase) per tile by contiguous pairwise halving,
            # in place in gbuf: view as [p, t, W, C]; each level writes
            # [t, i] from [t, 2i]+[t, 2i+1] (writes trail reads), W -> W/2.
            agg = apool.tile([P, NT * C], f32, tag="agg")
            curw = 4 * KG
            while curw > 1:
                srcv = gbuf[:, : nt * curw * C].rearrange(
                    "p (t w c) -> p t w c", t=nt, c=C
                )
                if curw & 1:
                    # fold the odd leftover into column 0 (write trails reads)
                    nc.vector.tensor_tensor(
                        out=srcv[:, :, 0, :],
                        in0=srcv[:, :, 0, :],
                        in1=srcv[:, :, curw - 1, :],
                        op=mybir.AluOpType.add,
                    )
                    curw -= 1
                H = curw // 2
                dstv = gbuf[:, : nt * H * C].rearrange(
                    "p (t w c) -> p t w c", t=nt, c=C
                )
                nc.vector.tensor_tensor(
                    out=dstv[:, :, :H, :],
                    in0=srcv[:, :, 0 : 2 * H : 2, :],
                    in1=srcv[:, :, 1 : 2 * H : 2, :],
                    op=mybir.AluOpType.add,
                )
                curw = H
            nc.vector.tensor_copy(out=agg[:, : nt * C], in_=gbuf[:, : nt * C])
            # add self-loop term
            nc.vector.tensor_add(
                out=agg[:, : nt * C],
                in0=agg[:, : nt * C],
                in1=xs_all[:, g0 * C : g1 * C],
            )
            # dinv scale (per node row)
            nc.vector.tensor_tensor(
                out=agg[:, : nt * C].rearrange("p (t c) -> p t c", c=C),
                in0=agg[:, : nt * C].rearrange("p (t c) -> p t c", c=C),
                in1=dinv_sh[:, g0:g1].to_broadcast([P, nt, C]),
                op=mybir.AluOpType.mult,
            )
            # batched transpose + block-diag matmul
            trps = pst.tile([P, P], f32, tag="trps")
            nc.tensor.transpose(
                out=trps[: nt * C, :], in_=agg[:, : nt * C], identity=ident[:]
            )
            aggdT = tspool.tile([P, P], f32, tag="aggdT")
            nc.scalar.copy(out=aggdT[: nt * C, :], in_=trps[: nt * C, :])
            mm = psm.tile([P, P], f32, tag="mm")
            nc.tensor.matmul(
                out=mm[:, : nt * C],
                lhsT=aggdT[: nt * C, :],
                rhs=w4a[: nt * C, : nt * C],
                start=True, stop=True,
            )
            osl = out_all[:, g0 * C : g1 * C]
            if bias_zero:
                nc.scalar.activation(
                    out=osl, in_=mm[:, : nt * C],
                    func=mybir.ActivationFunctionType.Relu,
                )
            else:
                for ti in range(nt):
                    nc.vector.tensor_add(
                        out=osl[:, ti * C : (ti + 1) * C],
                        in0=mm[:, ti * C : (ti + 1) * C],
                        in1=bsum_sb[:],
                    )
                nc.scalar.activation(
                    out=osl, in_=osl, func=mybir.ActivationFunctionType.Relu
                )

        nc.sync.dma_start(out=out_d.ap(), in_=out_all[:])

    nc.compile()
    return nc


_CACHE = {}


def _get_program(groups, calls, SG, TOT16, bias_zero):
    key = (tuple(groups), tuple(calls), SG, TOT16, bias_zero)
    if key not in _CACHE:
        _CACHE[key] = _build_program(groups, calls, SG, TOT16, bias_zero)
    return _CACHE[key]


def run(x, edge_index, W1, b1, W2, b2, trace=False):
    prep = _host_prep(x, edge_index)
    bias_zero = not (np.any(np.asarray(b1)) or np.any(np.asarray(b2)))
    nc = _get_program(prep["groups"], prep["calls"], prep["SG"], prep["TOT16"],
                      bias_zero)

    W1 = np.ascontiguousarray(np.asarray(W1, np.float32))
    W2 = np.ascontiguousarray(np.asarray(W2, np.float32))
    b1 = np.ascontiguousarray(np.asarray(b1, np.float32))
    b2 = np.ascontiguousarray(np.asarray(b2, np.float32))

    in_maps = []
    for c in range(M):
        in_maps.append({
            "x_pad": prep["x_pad"],
            "deg_arr": prep["deg_arr"],
            "deg_shard": np.ascontiguousarray(prep["deg_shard"][c]),
            "x_shard": np.ascontiguousarray(prep["x_shard"][c].reshape(P, TILES * C)),
            "idx16": np.ascontiguousarray(prep["idx16"][c]),
            "mask": np.ascontiguousarray(prep["mask"][c]),
            "W1": W1, "W2": W2, "b1": b1, "b2": b2,
        })

    res = run_bass_kernel_spmd(nc, in_maps, core_ids=list(range(M)), trace=trace)

    outs = np.stack(
        [res.results[c]["out"].reshape(P, TILES, C) for c in range(M)]
    )  # [M, P, TILES, C]
    t_of = prep["pos_of"] >> 7
    p_of = prep["pos_of"] & 127
    full = outs[prep["core_of"], p_of, t_of]
    return np.ascontiguousarray(full, dtype=np.float32), res


def kernel(x, edge_index, W1, b1, W2, b2):
    out, _ = run(x, edge_index, W1, b1, W2, b2, trace=False)
    return out


# revision 13
# speedup vs baseline: 1.5859x; 1.1589x over previous
"""Trainium2 Bass kernel for nn_DiffusionGraphConv_89936615178296.

out = relu(A_hat @ (x @ (W1+W2)) + b1 + b2), A_hat = D^-1/2 (A+I) D^-1/2.

Reformulation: out = relu(dinv * (Agg(dinv*x) @ W) + b), W = W1+W2,
dinv = rsqrt(1 + in_degree); the per-edge norm dinv[src]*dinv[dst] is
separable so per-edge work is a pure gather + segment-sum of x' = dinv*x.

8 cores, no collectives. Nodes sharded round-robin in degree-sorted order.
Each core:
  P0: builds the fp16 table x' = dinv*x for the full graph in its local HBM.
  P1: per destination tile group, one batched dma_gather (InstDMAGatherAnt,
      int16 element indices = src>>2, each 256B element = 4 packed fp16 rows)
      pulls all slots into a [128, cols, 128] grid; a host-built one-hot
      phase mask (src&3) zeroes the 3 unwanted rows per slot; one 4D-AP
      tensor_reduce folds (slot, phase) per tile; dinv scale; batched
      transpose + block-diag matmul with W on the tensor engine; relu;
      final DMA writes the output shard.

Host does integer index prep only (degree counts, shard permutation, slot
layout, int16 gather indices, 0/1 phase masks); all f32 arithmetic runs on
device.
"""

import numpy as np
from contextlib import ExitStack

import concourse.bass as bass
import concourse.bacc as bacc
import concourse.tile as tile
from concourse import mybir
from concourse.masks import make_identity
from concourse.bass_utils import run_bass_kernel_spmd

N, E, C = 100000, 1600000, 32
M = 8
P = 128
TILES = 98                  # 98*128 = 12544 rows per core >= 12500
TABLE_STRIPS = 783          # 783*128 = 100224 >= N+4 (zero rows 100000..100003)
TABLE_ROWS = TABLE_STRIPS * P
ZERO_ELEM = 25000           # element covering rows 100000..100003 (all zero)
NELEM = TABLE_ROWS * C // 128  # gatherable 256B elements in the table
G_STRIP = 64                # table build strip width
NT = 4                      # tiles per group
CALL_COLS = 48              # max grid columns per dma_gather call (6144 idxs)
GROUP_COLS = 96             # max grid columns per group (grid tile = 24KB/partition)


def _host_prep(x, edge_index):
    src = np.asarray(edge_index[0], dtype=np.int64)
    dst = np.asarray(edge_index[1], dtype=np.int64)
    x = np.asarray(x, dtype=np.float32)

    deg = np.bincount(dst, minlength=N).astype(np.int64) + 1

    order = np.argsort(-deg, kind="stable")
    ar = np.arange(N)
    core_of = np.empty(N, np.int64)
    pos_of = np.empty(N, np.int64)
    core_of[order] = ar % M
    pos_of[order] = ar // M

    degs_sorted = deg[order]
    # per-tile edge-slot count (self-loop handled via x_shard), >= 1
    Kt = [int(degs_sorted[t * P * M]) - 1 if t * P * M < N else 1 for t in range(TILES)]
    Kt = [max(1, k) for k in Kt]

    # groups of up to NT tiles with uniform K = max tile K in group;
    # cap group width (KG * nt) at GROUP_COLS so 3 grid buffers fit in SBUF
    groups = []  # (g0, g1, KG, colbase)
    col = 0
    g0 = 0
    while g0 < TILES:
        KG = max(1, Kt[g0])
        nt = max(1, min(NT, GROUP_COLS // KG))
        g1 = min(g0 + nt, TILES)
        KG = max(Kt[g0:g1])
        groups.append((g0, g1, KG, col))
        col += KG * (g1 - g0)
        g0 = g1
    SG = col  # total grid columns per core

    # gather calls: split groups at CALL_COLS boundaries
    # call = (col_start, ncols, o16) ; o16 = offset into idx16 array cols
    calls = []
    o16 = 0
    for (g0, g1, KG, cb) in groups:
        ncols_g = KG * (g1 - g0)
        cs = 0
        while cs < ncols_g:
            cc = min(CALL_COLS, ncols_g - cs)
            calls.append((cb + cs, cc, o16))
            o16 += cc * P // 16
            cs += cc
    TOT16 = o16

    t_of = pos_of >> 7
    p_of = pos_of & 127

    # global column of tile t slot k:  colbase(group) + (t-g0)*KG + k
    KG_of_t = np.empty(TILES, np.int64)
    colbase_of_t = np.empty(TILES, np.int64)
    for (g0, g1, KG, cb) in groups:
        for t in range(g0, g1):
            KG_of_t[t] = KG
            colbase_of_t[t] = cb + (t - g0) * KG

    # CSR by destination: rank of edge within its dst
    eorder = np.argsort(dst, kind="stable")
    src_sorted = src[eorder]
    dstn = dst[eorder]
    starts = np.concatenate([[0], np.cumsum(deg - 1)])
    rank = np.arange(E, dtype=np.int64) - starts[dstn]

    ecore = core_of[dstn]
    gcol = colbase_of_t[t_of[dstn]] + rank
    flat = gcol * P + p_of[dstn]            # slot id within core, col-major

    # int16 element indices (src>>2), pad = ZERO_ELEM
    A = np.full((M, SG * P), ZERO_ELEM, np.int16)
    A[ecore, flat] = (src_sorted >> 2).astype(np.int16)
    # phase mask fp16 [M, P, 4*SG]: 1.0 at (p, 4*gcol + (src&3))
    mask = np.zeros((M, P, 4 * SG), np.float16)
    mask[ecore, p_of[dstn], 4 * gcol + (src_sorted & 3)] = 1.0

    # wrap each call's indices: [16, n/16] with [l, s] = arr[s*16+l], tile x8
    idx16 = np.zeros((M, P, TOT16), np.int16)
    for (c0, cc, o16c) in calls:
        n = cc * P
        sl = A[:, c0 * P : c0 * P + n]                       # [M, n]
        wr = sl.reshape(M, n // 16, 16).transpose(0, 2, 1)   # [M, 16, n/16]
        idx16[:, :, o16c : o16c + n // 16] = np.tile(wr, (1, 8, 1))

    # per-core own-shard features, [P, TILES, C], zero on pad rows
    x_shard = np.zeros((M, P, TILES, C), np.float32)
    x_shard[core_of, p_of, t_of] = x

    deg_pad = np.ones(TABLE_ROWS, np.float32)
    deg_pad[:N] = deg
    deg_arr = np.ones((P, TABLE_STRIPS), np.float32)
    colpos = 0
    for s0 in range(0, TABLE_STRIPS, G_STRIP):
        g = min(G_STRIP, TABLE_STRIPS - s0)
        blk = deg_pad[s0 * P : (s0 + g) * P].reshape(P, g)
        deg_arr[:, colpos : colpos + g] = blk
        colpos += g

    deg_shard = np.ones((M, P, TILES), np.float32)
    deg_shard[core_of, p_of, t_of] = deg.astype(np.float32)

    x_pad = np.zeros((TABLE_ROWS, C), np.float32)
    x_pad[:N] = x

    return dict(
        groups=groups, calls=calls, SG=SG, TOT16=TOT16, idx16=idx16,
        mask=mask, deg_arr=deg_arr, deg_shard=deg_shard, x_pad=x_pad,
        core_of=core_of, pos_of=pos_of, x_shard=x_shard,
    )


def _build_program(groups, calls, SG, TOT16, bias_zero):
    nc = bacc.Bacc("TRN2", target_bir_lowering=False, debug=False, num_devices=M)
    f32, f16, i16 = mybir.dt.float32, mybir.dt.float16, mybir.dt.int16

    x_d = nc.dram_tensor("x_pad", [TABLE_ROWS, C], f32, kind="ExternalInput")
    dega_d = nc.dram_tensor("deg_arr", [P, TABLE_STRIPS], f32, kind="ExternalInput")
    degs_d = nc.dram_tensor("deg_shard", [P, TILES], f32, kind="ExternalInput")
    idx_d = nc.dram_tensor("idx16", [P, TOT16], i16, kind="ExternalInput")
    mask_d = nc.dram_tensor("mask", [P, 4 * SG], f16, kind="ExternalInput")
    xs_d = nc.dram_tensor("x_shard", [P, TILES * C], f32, kind="ExternalInput")
    w1_d = nc.dram_tensor("W1", [C, C], f32, kind="ExternalInput")
    w2_d = nc.dram_tensor("W2", [C, C], f32, kind="ExternalInput")
    b1_d = nc.dram_tensor("b1", [C], f32, kind="ExternalInput")
    b2_d = nc.dram_tensor("b2", [C], f32, kind="ExternalInput")
    table_d = nc.dram_tensor("table", [TABLE_ROWS, C], f16, kind="Internal")
    out_d = nc.dram_tensor("out", [P, TILES * C], f32, kind="ExternalOutput")

    with tile.TileContext(nc) as tc, ExitStack() as ctx:
        singles = ctx.enter_context(tc.tile_pool(name="singles", bufs=1))
        xpool = ctx.enter_context(tc.tile_pool(name="xin", bufs=3))
        tpool = ctx.enter_context(tc.tile_pool(name="tout", bufs=3))
        gpool = ctx.enter_context(tc.tile_pool(name="gather", bufs=4))
        apool = ctx.enter_context(tc.tile_pool(name="agg", bufs=3))
        tspool = ctx.enter_context(tc.tile_pool(name="trsb", bufs=3))
        pst = ctx.enter_context(tc.tile_pool(name="pst", bufs=3, space="PSUM"))
        psm = ctx.enter_context(tc.tile_pool(name="psm", bufs=3, space="PSUM"))

        # ---- singles ----
        w4a = singles.tile([P, P], f32)
        w4b = singles.tile([P, P], f32)
        nc.vector.memset(w4a[:], 0.0)
        nc.gpsimd.memset(w4b[:], 0.0)
        for t in range(NT):
            sl = slice(t * C, (t + 1) * C)
            nc.sync.dma_start(out=w4a[sl, sl], in_=w1_d.ap())
            nc.sync.dma_start(out=w4b[sl, sl], in_=w2_d.ap())
        nc.vector.tensor_add(out=w4a[:], in0=w4a[:], in1=w4b[:])

        if not bias_zero:
            b1_sb = singles.tile([P, C], f32)
            b2_sb = singles.tile([P, C], f32)
            nc.sync.dma_start(
                out=b1_sb[:], in_=bass.AP(tensor=b1_d, offset=0, ap=[[0, P], [1, C]])
            )
            nc.sync.dma_start(
                out=b2_sb[:], in_=bass.AP(tensor=b2_d, offset=0, ap=[[0, P], [1, C]])
            )
            bsum_sb = singles.tile([P, C], f32)
            nc.vector.tensor_add(out=bsum_sb[:], in0=b1_sb[:], in1=b2_sb[:])

        ident = singles.tile([P, P], f32)
        make_identity(nc, ident[:])

        dinv_all = singles.tile([P, TABLE_STRIPS], f32)
        nc.sync.dma_start(out=dinv_all[:], in_=dega_d.ap())
        nc.scalar.sqrt(out=dinv_all[:], in_=dinv_all[:])
        nc.vector.reciprocal(out=dinv_all[:], in_=dinv_all[:])

        dinv_sh = singles.tile([P, TILES], f32)
        nc.sync.dma_start(out=dinv_sh[:], in_=degs_d.ap())
        nc.scalar.sqrt(out=dinv_sh[:], in_=dinv_sh[:])
        nc.vector.reciprocal(out=dinv_sh[:], in_=dinv_sh[:])

        out_all = singles.tile([P, TILES * C], f32)

        # ---- P0: table build, partition-contiguous strips ----
        colpos = 0
        for s0 in range(0, TABLE_STRIPS, G_STRIP):
            g = min(G_STRIP, TABLE_STRIPS - s0)
            x_ap = bass.AP(
                tensor=x_d, offset=s0 * P * C,
                ap=[[g * C, P], [C, g], [1, C]],
            )
            t_ap = bass.AP(
                tensor=table_d, offset=s0 * P * C,
                ap=[[g * C, P], [C, g], [1, C]],
            )
            x_sb = xpool.tile([P, G_STRIP, C], f32, tag="x")
            nc.sync.dma_start(out=x_sb[:, :g, :], in_=x_ap)
            t_sb = tpool.tile([P, G_STRIP, C], f16, tag="t")
            nc.vector.tensor_tensor(
                out=t_sb[:, :g, :],
                in0=x_sb[:, :g, :],
                in1=dinv_all[:, colpos : colpos + g].to_broadcast([P, g, C]),
                op=mybir.AluOpType.mult,
            )
            nc.sync.dma_start(out=t_ap, in_=t_sb[:, :g, :])
            colpos += g

        # deferred singles: loaded after the table strips so P0 gets
        # full DMA bandwidth; all are ready well before their consumers
        idx_sb = singles.tile([P, TOT16], i16)
        nc.sync.dma_start(out=idx_sb[:], in_=idx_d.ap())
        mask_sb = singles.tile([P, 4 * SG], f16)
        nc.sync.dma_start(out=mask_sb[:], in_=mask_d.ap())
        xs_all = singles.tile([P, TILES * C], f32)
        nc.sync.dma_start(out=xs_all[:], in_=xs_d.ap())
        nc.vector.tensor_tensor(
            out=xs_all[:].rearrange("p (t c) -> p t c", c=C),
            in0=xs_all[:].rearrange("p (t c) -> p t c", c=C),
            in1=dinv_sh[:].to_broadcast([P, TILES, C]),
            op=mybir.AluOpType.mult,
        )

        # gather source: 256B elements = 4 packed fp16 rows
        table_elems = bass.AP(tensor=table_d, offset=0, ap=[[128, NELEM], [1, 128]])

        # ---- P1 ----
        maxcols = max(KG * (g1 - g0) for g0, g1, KG, _ in groups)
        call_i = 0
        for (g0, g1, KG, cb) in groups:
            nt = g1 - g0
            ncols = KG * nt
            gbuf = gpool.tile([P, maxcols * 128], f16, tag="gbuf")
            # one (or few) batched gathers for the whole group
            cdone = 0
            while cdone < ncols:
                c0, cc, o16c = calls[call_i]
                assert c0 == cb + cdone, (c0, cb, cdone)
                nc.gpsimd.dma_gather(
                    out_ap=gbuf[:, cdone * 128 : (cdone + cc) * 128].rearrange(
                        "p (g e) -> p g e", e=128
                    ),
                    in_ap=table_elems,
                    idxs_ap=idx_sb[:, o16c : o16c + cc * P // 16],
                    num_idxs=cc * P,
                    num_idxs_reg=cc * P,
                    elem_size=128,
                    elem_step=128,
                    single_packet=False,
                )
                call_i += 1
                cdone += cc
            # phase select: grid *= one-hot mask over the 4 rows per element
            nc.vector.tensor_tensor(
                out=gbuf[:, : ncols * 128].rearrange(
                    "p (s m c) -> p s m c", m=4, c=C
                ),
                in0=gbuf[:, : ncols * 128].rearrange(
                    "p (s m c) -> p s m c", m=4, c=C
                ),
                in1=mask_sb[:, 4 * cb : 4 * (cb + ncols)]
                .rearrange("p (s m) -> p s m", m=4)
                .to_broadcast([P, ncols, 4, C]),
                op=mybir.AluOpType.mult,
            )
            # fold (slot, phase) per tile by contiguous pairwise halving,
            # in place in gbuf: view as [p, t, W, C]; each level writes
            # [t, i] from [t, 2i]+[t, 2i+1] (writes trail reads), W -> W/2.
            agg = apool.tile([P, NT * C], f32, tag="agg")
            curw = 4 * KG
            while curw > 1:
                srcv = gbuf[:, : nt * curw * C].rearrange(
                    "p (t w c) -> p t w c", t=nt, c=C
                )
                if curw & 1:
                    # fold the odd leftover into column 0 (write trails reads)
                    nc.vector.tensor_tensor(
                        out=srcv[:, :, 0, :],
                        in0=srcv[:, :, 0, :],
                        in1=srcv[:, :, curw - 1, :],
                        op=mybir.AluOpType.add,
                    )
                    curw -= 1
                H = curw // 2
                dstv = gbuf[:, : nt * H * C].rearrange(
                    "p (t w c) -> p t w c", t=nt, c=C
                )
                nc.vector.tensor_tensor(
                    out=dstv[:, :, :H, :],
                    in0=srcv[:, :, 0 : 2 * H : 2, :],
                    in1=srcv[:, :, 1 : 2 * H : 2, :],
                    op=mybir.AluOpType.add,
                )
                curw = H
            nc.vector.tensor_copy(out=agg[:, : nt * C], in_=gbuf[:, : nt * C])
            # add self-loop term
            nc.vector.tensor_add(
                out=agg[:, : nt * C],
                in0=agg[:, : nt * C],
                in1=xs_all[:, g0 * C : g1 * C],
            )
            # dinv scale (per node row)
            nc.vector.tensor_tensor(
                out=agg[:, : nt * C].rearrange("p (t c) -> p t c", c=C),
                in0=agg[:, : nt * C].rearrange("p (t c) -> p t c", c=C),
                in1=dinv_sh[:, g0:g1].to_broadcast([P, nt, C]),
                op=mybir.AluOpType.mult,
            )
            # batched transpose + block-diag matmul
            trps = pst.tile([P, P], f32, tag="trps")
            nc.tensor.transpose(
                out=trps[: nt * C, :], in_=agg[:, : nt * C], identity=ident[:]
            )
            aggdT = tspool.tile([P, P], f32, tag="aggdT")
            nc.scalar.copy(out=aggdT[: nt * C, :], in_=trps[: nt * C, :])
            mm = psm.tile([P, P], f32, tag="mm")
            nc.tensor.matmul(
                out=mm[:, : nt * C],
                lhsT=aggdT[: nt * C, :],
                rhs=w4a[: nt * C, : nt * C],
                start=True, stop=True,
            )
            osl = out_all[:, g0 * C : g1 * C]
            if bias_zero:
                nc.scalar.activation(
                    out=osl, in_=mm[:, : nt * C],
                    func=mybir.ActivationFunctionType.Relu,
                )
            else:
                for ti in range(nt):
                    nc.vector.tensor_add(
                        out=osl[:, ti * C : (ti + 1) * C],
                        in0=mm[:, ti * C : (ti + 1) * C],
                        in1=bsum_sb[:],
                    )
                nc.scalar.activation(
                    out=osl, in_=osl, func=mybir.ActivationFunctionType.Relu
                )

        nc.sync.dma_start(out=out_d.ap(), in_=out_all[:])

    nc.compile()
    return nc


_CACHE = {}


def _get_program(groups, calls, SG, TOT16, bias_zero):
    key = (tuple(groups), tuple(calls), SG, TOT16, bias_zero)
    if key not in _CACHE:
        _CACHE[key] = _build_program(groups, calls, SG, TOT16, bias_zero)
    return _CACHE[key]


def run(x, edge_index, W1, b1, W2, b2, trace=False):
    prep = _host_prep(x, edge_index)
    bias_zero = not (np.any(np.asarray(b1)) or np.any(np.asarray(b2)))
    nc = _get_program(prep["groups"], prep["calls"], prep["SG"], prep["TOT16"],
                      bias_zero)

    W1 = np.ascontiguousarray(np.asarray(W1, np.float32))
    W2 = np.ascontiguousarray(np.asarray(W2, np.float32))
    b1 = np.ascontiguousarray(np.asarray(b1, np.float32))
    b2 = np.ascontiguousarray(np.asarray(b2, np.float32))

    in_maps = []
    for c in range(M):
        in_maps.append({
            "x_pad": prep["x_pad"],
            "deg_arr": prep["deg_arr"],
            "deg_shard": np.ascontiguousarray(prep["deg_shard"][c]),
            "x_shard": np.ascontiguousarray(prep["x_shard"][c].reshape(P, TILES * C)),
            "idx16": np.ascontiguousarray(prep["idx16"][c]),
            "mask": np.ascontiguousarray(prep["mask"][c]),
            "W1": W1, "W2": W2, "b1": b1, "b2": b2,
        })

    res = run_bass_kernel_spmd(nc, in_maps, core_ids=list(range(M)), trace=trace)

    outs = np.stack(
        [res.results[c]["out"].reshape(P, TILES, C) for c in range(M)]
    )  # [M, P, TILES, C]
    t_of = prep["pos_of"] >> 7
    p_of = prep["pos_of"] & 127
    full = outs[prep["core_of"], p_of, t_of]
    return np.ascontiguousarray(full, dtype=np.float32), res


def kernel(x, edge_index, W1, b1, W2, b2):
    out, _ = run(x, edge_index, W1, b1, W2, b2, trace=False)
    return out
